# revision 21
# baseline (speedup 1.0000x reference)
"""Causal multi-head attention block on 8 NeuronCores (Trainium2, Bass/Tile).

Reference computation (per batch b):
  Q = x @ W_Q + b_Q ; K = x @ W_K + b_K ; V = x @ W_V + b_V   (per head)
  scores = Q K^T / sqrt(H); causal mask; probs = softmax(scores)
  out = (probs @ V) @ W_O + b_O

Sharding: core c -> batch c//2, head-group c%2 (6 of 12 heads).
Each core computes a partial output [S, D] (its heads' contribution,
with b_Q/b_K applied on-device). Host sums the two head-group partials
per batch and adds b_O + sum_nh b_V[n,h] * W_O[n,h,:] (exact: the b_V
term factors out because softmax rows sum to 1).

Device-side layout choices:
  - x arrives pre-transposed from the host (xT: [d, s]) since both
    projection operands need the contraction dim (d) on partitions.
  - Q^T, K^T produced directly as [h, s] (head pairs stacked to 128
    partitions for full PE utilization).
  - scores are computed transposed ([k, q]) so that the softmax sum over k
    can be taken by a matmul: V is augmented with a ones column, making the
    PV matmul emit both z^T (64 rows) and the softmax denominator (row 64).
  - softmax skips max-subtraction (scores are O(1) for this distribution;
    exp is computed on the raw scaled scores).
  - causal handling: fully-masked tiles skipped; on diagonal tiles scores/
    exp/PV only touch the live column range; the single shared 128x128
    upper-triangular mask handles the partial block.
  - all matmuls run with operands bitcast to float32r (fp32 stored, fp22
    multiplied) - full PE rate when the moving free dim >= 256.
"""

import sys

sys.path.insert(0, "/opt/trn_rl_repo")

from contextlib import ExitStack

import numpy as np

import concourse.bass as bass
import concourse.tile as tile
from concourse import bacc, mybir
from concourse.bass_utils import run_bass_kernel_spmd

B, S, D, N, H = 4, 1024, 768, 12, 64
NHC = 6            # heads per core
NPAIR = NHC // 2   # head pairs per core (2 heads stacked -> 128 partitions)
HD = NHC * H       # 384: per-core packed head dim
P = 128
NDT = D // P       # 6 d-tiles
NST = S // P       # 8 s-tiles (also k-tiles)
QB = 512           # q block (moving-dim tile for most matmuls)
NQB = S // QB      # 2
F32 = mybir.dt.float32
F32R = mybir.dt.float32r
EXP_SCALE = 1.0 / np.sqrt(float(H))

_CACHE = {}


def _r(ap):
    """Bitcast an fp32 AP to float32r for full-rate PE matmuls."""
    return ap.bitcast(F32R)


def _build():
    nc = bacc.Bacc()
    xt_d = nc.declare_dram_parameter("xt", [D, S], F32, isOutput=False)
    wq_d = nc.declare_dram_parameter("wq", [D, HD], F32, isOutput=False)
    wk_d = nc.declare_dram_parameter("wk", [D, HD], F32, isOutput=False)
    wv_d = nc.declare_dram_parameter("wv", [D, HD], F32, isOutput=False)
    wo_d = nc.declare_dram_parameter("wo", [HD, D], F32, isOutput=False)
    bq_d = nc.declare_dram_parameter("bq", [P, NPAIR], F32, isOutput=False)
    bk_d = nc.declare_dram_parameter("bk", [P, NPAIR], F32, isOutput=False)
    tri_d = nc.declare_dram_parameter("trimask", [P, P], F32, isOutput=False)
    out_d = nc.declare_dram_parameter("out", [S, D], F32, isOutput=True)

    xt_r = xt_d[:].bitcast(F32R).rearrange("(t p) s -> p t s", p=P)
    wq_r = wq_d[:].bitcast(F32R).rearrange("(t p) h -> p t h", p=P)
    wk_r = wk_d[:].bitcast(F32R).rearrange("(t p) h -> p t h", p=P)
    wv_r = wv_d[:].bitcast(F32R).rearrange("(t p) h -> p t h", p=P)
    wo_r = wo_d[:].bitcast(F32R).rearrange("(t p) d -> p t d", p=P)

    with tile.TileContext(nc) as tc, ExitStack() as ctx:
        consts = ctx.enter_context(tc.tile_pool(name="consts", bufs=1))
        persist = ctx.enter_context(tc.tile_pool(name="persist", bufs=1))
        etp = ctx.enter_context(tc.tile_pool(name="etp", bufs=4))
        smalls = ctx.enter_context(tc.tile_pool(name="smalls", bufs=4))
        outp = ctx.enter_context(tc.tile_pool(name="outp", bufs=3))

        # ---- DMA emission order == priority order on the shared DMA device.
        # qb0 attention needs only the s2=0 halves of x^T/Q^T/K^T and the
        # first 4 V k-tiles, so those stream in first.
        xT = consts.tile([P, NDT, S], F32)
        wq_sb = consts.tile([P, NDT, HD], F32)
        wk_sb = consts.tile([P, NDT, HD], F32)
        wv_sb = consts.tile([P, NDT, HD], F32)
        bq_sb = consts.tile([P, NPAIR], F32)
        bk_sb = consts.tile([P, NPAIR], F32)
        tri = consts.tile([P, P], F32)
        wo_sb = consts.tile([P, NPAIR, D], F32)

        def load_xt(dt_, s2):
            nc.sync.dma_start(
                out=xT[:, dt_, s2 * QB : (s2 + 1) * QB].bitcast(F32R),
                in_=xt_r[:, dt_, s2 * QB : (s2 + 1) * QB],
            )

        def load_w_cols(w_sb, w_r, g):
            nc.sync.dma_start(
                out=w_sb[:, :, g * P : (g + 1) * P].bitcast(F32R),
                in_=w_r[:, :, g * P : (g + 1) * P],
            )

        # DMA priority order: enable pair-0 s2=0 projections asap, then V,
        # then the later pairs, then everything qb1 needs.
        load_w_cols(wq_sb, wq_r, 0)
        load_w_cols(wk_sb, wk_r, 0)
        nc.sync.dma_start(out=bq_sb, in_=bq_d[:])
        nc.sync.dma_start(out=bk_sb, in_=bk_d[:])
        nc.sync.dma_start(out=tri, in_=tri_d[:])
        for dt_ in range(NDT):
            load_xt(dt_, 0)
        nc.sync.dma_start(out=wv_sb[:].bitcast(F32R), in_=wv_r)
        load_w_cols(wq_sb, wq_r, 1)
        load_w_cols(wk_sb, wk_r, 1)
        load_w_cols(wq_sb, wq_r, 2)
        load_w_cols(wk_sb, wk_r, 2)
        nc.sync.dma_start(
            out=xT[:, :, QB:S].bitcast(F32R), in_=xt_r[:, :, QB:S]
        )
        nc.sync.dma_start(out=wo_sb[:].bitcast(F32R), in_=wo_r)

        # ---- persistent activations ----
        qT = persist.tile([P, NPAIR, S], F32)     # Q^T, head pairs stacked
        kT = persist.tile([P, NPAIR, S], F32)
        vA = persist.tile([P, NST, NHC, H + 1], F32)  # V + ones col, per k-tile
        zT = persist.tile([P, NPAIR, S], F32)     # z^T (normalized), pairs stacked

        nc.gpsimd.memset(vA[:, :, :, H : H + 1], 1.0)

        ps_proj = ctx.enter_context(tc.tile_pool(name="ps_proj", bufs=2, space="PSUM"))
        ps_s = ctx.enter_context(tc.tile_pool(name="ps_s", bufs=1, space="PSUM"))
        ps_z = ctx.enter_context(tc.tile_pool(name="ps_z", bufs=1, space="PSUM"))
        ps_o = ctx.enter_context(tc.tile_pool(name="ps_o", bufs=2, space="PSUM"))

        def proj_qk(g, s2):
            qps = ps_proj.tile([P, QB], F32, tag="qk")
            for dt_ in range(NDT):
                nc.tensor.matmul(
                    qps,
                    _r(wq_sb[:, dt_, g * P : (g + 1) * P]),
                    _r(xT[:, dt_, s2 * QB : (s2 + 1) * QB]),
                    start=(dt_ == 0),
                    stop=(dt_ == NDT - 1),
                )
            nc.scalar.add(
                qT[:, g, s2 * QB : (s2 + 1) * QB].bitcast(F32R),
                qps,
                bq_sb[:, g : g + 1],
            )
            kps = ps_proj.tile([P, QB], F32, tag="qk")
            for dt_ in range(NDT):
                nc.tensor.matmul(
                    kps,
                    _r(wk_sb[:, dt_, g * P : (g + 1) * P]),
                    _r(xT[:, dt_, s2 * QB : (s2 + 1) * QB]),
                    start=(dt_ == 0),
                    stop=(dt_ == NDT - 1),
                )
            nc.scalar.add(
                kT[:, g, s2 * QB : (s2 + 1) * QB].bitcast(F32R),
                kps,
                bk_sb[:, g : g + 1],
            )

        def proj_v(st):
            vps = ps_proj.tile([P, HD], F32, tag="qk")
            for dt_ in range(NDT):
                nc.tensor.matmul(
                    vps,
                    _r(xT[:, dt_, st * P : (st + 1) * P]),
                    _r(wv_sb[:, dt_, :]),
                    start=(dt_ == 0),
                    stop=(dt_ == NDT - 1),
                )
            nc.vector.tensor_copy(
                out=vA[:, st, :, 0:H].bitcast(F32R),
                in_=vps.rearrange("p (n h) -> p n h", n=NHC),
            )

        def attend_pair(g, qb):
            """Both heads of pair g: the two K=64 score matmuls are packed
            into disjoint row-strips of the PE array via tile_position, so
            they run concurrently on the 32x32 sub-arrays."""
            q0 = qb * QB
            nkt = (qb + 1) * QB // P  # causal: k-tiles 0..nkt-1
            zzps = [
                ps_z.tile([H + 1, QB], F32, name=f"zps{hh}", tag=f"z{hh}")
                for hh in range(2)
            ]
            for kt in range(nkt):
                o = max(kt * P - q0, 0)  # first live column
                ets = []
                for hh in range(2):
                    hp = hh * H
                    sps = ps_s.tile([P, QB], F32, name=f"sps{hh}", tag=f"s{hh}")
                    nc.tensor.matmul(
                        sps[:, o:QB],
                        _r(kT[hp : hp + H, g, kt * P : (kt + 1) * P]),
                        _r(qT[hp : hp + H, g, q0 + o : q0 + QB]),
                        start=True,
                        stop=True,
                        tile_position=(hp, 0),
                    )
                    et = etp.tile([P, QB], F32)
                    nc.scalar.activation(
                        et[:, o:QB].bitcast(F32R),
                        sps[:, o:QB],
                        mybir.ActivationFunctionType.Exp,
                        scale=EXP_SCALE,
                    )
                    if kt * P - q0 >= -(P - 1):  # diagonal tile: partial block
                        nc.vector.tensor_mul(
                            et[:, o : o + P].bitcast(F32R), et[:, o : o + P], tri
                        )
                    ets.append(et)
                for hh in range(2):
                    nc.tensor.matmul(
                        zzps[hh][:, o:QB],
                        _r(vA[:, kt, 2 * g + hh, :]),
                        _r(ets[hh][:, o:QB]),
                        start=(kt == 0),
                        stop=(kt == nkt - 1),
                    )
            for hh in range(2):
                hp = hh * H
                zps = zzps[hh]
                # normalize: r = 1/l, broadcast over 64 partitions (gpsimd)
                r = smalls.tile([1, QB], F32)
                nc.vector.reciprocal(r, zps[H : H + 1, :])
                rb = smalls.tile([H, QB], F32, tag="rb")
                nc.gpsimd.partition_broadcast(rb, r)
                nc.vector.tensor_mul(
                    zT[hp : hp + H, g, q0 : q0 + QB].bitcast(F32R),
                    zps[0:H, :],
                    rb,
                )

        def out_proj(qb):
            q0 = qb * QB
            for qt in range(QB // P):
                row0 = q0 + qt * P
                out_t = outp.tile([P, D], F32)
                for dh in range(2):
                    ops = ps_o.tile([P, D // 2], F32)
                    for g in range(NPAIR):
                        nc.tensor.matmul(
                            ops,
                            _r(zT[:, g, row0 : row0 + P]),
                            _r(wo_sb[:, g, dh * (D // 2) : (dh + 1) * (D // 2)]),
                            start=(g == 0),
                            stop=(g == NPAIR - 1),
                        )
                    nc.vector.tensor_copy(
                        out=out_t[:, dh * (D // 2) : (dh + 1) * (D // 2)], in_=ops
                    )
                nc.sync.dma_start(out=out_d[row0 : row0 + P, :], in_=out_t)

        # phase 1+2: s2=0 projections pair-interleaved with qb0 attention
        proj_qk(0, 0)
        for st in range(4):
            proj_v(st)
        attend_pair(0, 0)
        proj_qk(1, 0)
        attend_pair(1, 0)
        proj_qk(2, 0)
        attend_pair(2, 0)
        out_proj(0)
        # phase 3: s2=1 projections, then qb1 attention
        for g in range(NPAIR):
            proj_qk(g, 1)
        for st in range(4, NST):
            proj_v(st)
        for g in range(NPAIR):
            attend_pair(g, 1)
        out_proj(1)

    if not nc.is_finalized():
        nc.finalize()
    return nc


def _get_program():
    if "nc" not in _CACHE:
        _CACHE["nc"] = _build()
    return _CACHE["nc"]


def make_in_maps(
    normalized_resid_pre, W_Q, W_K, W_V, W_O, b_Q, b_K, b_V=None, b_O=None, **_unused
):
    x = np.asarray(normalized_resid_pre, np.float32)
    W_Q, W_K, W_V = (np.asarray(a, np.float32) for a in (W_Q, W_K, W_V))
    W_O = np.asarray(W_O, np.float32)
    b_Q, b_K = np.asarray(b_Q, np.float32), np.asarray(b_K, np.float32)

    tri = np.triu(np.ones((P, P), np.float32))
    in_maps = []
    for c in range(8):
        b, hg = divmod(c, 2)
        hs = slice(hg * NHC, (hg + 1) * NHC)
        in_maps.append(
            {
                "xt": np.ascontiguousarray(x[b].T),
                "wq": np.ascontiguousarray(
                    W_Q[hs].transpose(1, 0, 2).reshape(D, HD)
                ),
                "wk": np.ascontiguousarray(
                    W_K[hs].transpose(1, 0, 2).reshape(D, HD)
                ),
                "wv": np.ascontiguousarray(
                    W_V[hs].transpose(1, 0, 2).reshape(D, HD)
                ),
                "wo": np.ascontiguousarray(W_O[hs].reshape(HD, D)),
                "bq": np.ascontiguousarray(b_Q[hs].reshape(NPAIR, P).T),
                "bk": np.ascontiguousarray(b_K[hs].reshape(NPAIR, P).T),
                "trimask": tri,
            }
        )
    return in_maps


def kernel(
    normalized_resid_pre, W_Q, W_K, W_V, W_O, b_Q, b_K, b_V, b_O, **_unused
):
    W_O = np.asarray(W_O, np.float32)
    b_V, b_O = np.asarray(b_V, np.float32), np.asarray(b_O, np.float32)
    in_maps = make_in_maps(
        normalized_resid_pre, W_Q, W_K, W_V, W_O, b_Q, b_K
    )

    nc = _get_program()
    res = run_bass_kernel_spmd(nc, in_maps, list(range(8))).results

    out = np.zeros((B, S, D), np.float32)
    for c in range(8):
        out[c // 2] += res[c]["out"]
    out += b_O + np.einsum("nh,nhd->d", b_V, W_O)
    return out


# revision 29
# speedup vs baseline: 1.0004x; 1.0004x over previous
"""Causal multi-head attention block on 8 NeuronCores (Trainium2, Bass/Tile).

Reference computation (per batch b):
  Q = x @ W_Q + b_Q ; K = x @ W_K + b_K ; V = x @ W_V + b_V   (per head)
  scores = Q K^T / sqrt(H); causal mask; probs = softmax(scores)
  out = (probs @ V) @ W_O + b_O

Sharding: core c -> batch c//2, head-group c%2 (6 of 12 heads).
Each core computes a partial output [S, D] (its heads' contribution,
with b_Q/b_K applied on-device). Host sums the two head-group partials
per batch and adds b_O + sum_nh b_V[n,h] * W_O[n,h,:] (exact: the b_V
term factors out because softmax rows sum to 1).

Device-side layout choices:
  - x arrives pre-transposed from the host (xT: [d, s]) since both
    projection operands need the contraction dim (d) on partitions.
  - Q^T, K^T produced directly as [h, s] (head pairs stacked to 128
    partitions for full PE utilization).
  - scores are computed transposed ([k, q]) so that the softmax sum over k
    can be taken by a matmul: V is augmented with a ones column, making the
    PV matmul emit both z^T (64 rows) and the softmax denominator (row 64).
  - softmax skips max-subtraction (scores are O(1) for this distribution;
    exp is computed on the raw scaled scores).
  - causal handling: fully-masked tiles skipped; on diagonal tiles scores/
    exp/PV only touch the live column range; the single shared 128x128
    upper-triangular mask handles the partial block.
  - all matmuls run with operands bitcast to float32r (fp32 stored, fp22
    multiplied) - full PE rate when the moving free dim >= 256.
"""

import sys

sys.path.insert(0, "/opt/trn_rl_repo")

from contextlib import ExitStack

import numpy as np

import concourse.bass as bass
import concourse.tile as tile
from concourse import bacc, mybir
from concourse.bass_utils import run_bass_kernel_spmd

B, S, D, N, H = 4, 1024, 768, 12, 64
NHC = 6            # heads per core
NPAIR = NHC // 2   # head pairs per core (2 heads stacked -> 128 partitions)
HD = NHC * H       # 384: per-core packed head dim
P = 128
NDT = D // P       # 6 d-tiles
NST = S // P       # 8 s-tiles (also k-tiles)
QB = 512           # q block (moving-dim tile for most matmuls)
NQB = S // QB      # 2
F32 = mybir.dt.float32
F32R = mybir.dt.float32r
EXP_SCALE = 1.0 / np.sqrt(float(H))

_CACHE = {}


def _r(ap):
    """Bitcast an fp32 AP to float32r for full-rate PE matmuls."""
    return ap.bitcast(F32R)


def _build():
    nc = bacc.Bacc()
    xt_d = nc.declare_dram_parameter("xt", [D, S], F32, isOutput=False)
    wq_d = nc.declare_dram_parameter("wq", [D, HD], F32, isOutput=False)
    wk_d = nc.declare_dram_parameter("wk", [D, HD], F32, isOutput=False)
    wv_d = nc.declare_dram_parameter("wv", [D, HD], F32, isOutput=False)
    wo_d = nc.declare_dram_parameter("wo", [HD, D], F32, isOutput=False)
    bq_d = nc.declare_dram_parameter("bq", [P, NPAIR], F32, isOutput=False)
    bk_d = nc.declare_dram_parameter("bk", [P, NPAIR], F32, isOutput=False)
    tri_d = nc.declare_dram_parameter("trimask", [P, P], F32, isOutput=False)
    out_d = nc.declare_dram_parameter("out", [S, D], F32, isOutput=True)

    xt_r = xt_d[:].bitcast(F32R).rearrange("(t p) s -> p t s", p=P)
    wq_r = wq_d[:].bitcast(F32R).rearrange("(t p) h -> p t h", p=P)
    wk_r = wk_d[:].bitcast(F32R).rearrange("(t p) h -> p t h", p=P)
    wv_r = wv_d[:].bitcast(F32R).rearrange("(t p) h -> p t h", p=P)
    wo_r = wo_d[:].bitcast(F32R).rearrange("(t p) d -> p t d", p=P)

    with tile.TileContext(nc) as tc, ExitStack() as ctx:
        consts = ctx.enter_context(tc.tile_pool(name="consts", bufs=1))
        persist = ctx.enter_context(tc.tile_pool(name="persist", bufs=1))
        etp = ctx.enter_context(tc.tile_pool(name="etp", bufs=4))
        smalls = ctx.enter_context(tc.tile_pool(name="smalls", bufs=4))
        outp = ctx.enter_context(tc.tile_pool(name="outp", bufs=3))

        # ---- DMA emission order == priority order on the shared DMA device.
        # qb0 attention needs only the s2=0 halves of x^T/Q^T/K^T and the
        # first 4 V k-tiles, so those stream in first.
        xT = consts.tile([P, NDT, S], F32)
        wq_sb = consts.tile([P, NDT, HD], F32)
        wk_sb = consts.tile([P, NDT, HD], F32)
        wv_sb = consts.tile([P, NDT, HD], F32)
        bq_sb = consts.tile([P, NPAIR], F32)
        bk_sb = consts.tile([P, NPAIR], F32)
        tri = consts.tile([P, P], F32)
        wo_sb = consts.tile([P, NPAIR, D], F32)

        def load_xt(dt_, s2):
            nc.sync.dma_start(
                out=xT[:, dt_, s2 * QB : (s2 + 1) * QB].bitcast(F32R),
                in_=xt_r[:, dt_, s2 * QB : (s2 + 1) * QB],
            )

        def load_w_cols(w_sb, w_r, g):
            nc.sync.dma_start(
                out=w_sb[:, :, g * P : (g + 1) * P].bitcast(F32R),
                in_=w_r[:, :, g * P : (g + 1) * P],
            )

        # DMA priority order: enable pair-0 s2=0 projections asap, then V,
        # then the later pairs, then everything qb1 needs.
        load_w_cols(wq_sb, wq_r, 0)
        load_w_cols(wk_sb, wk_r, 0)
        nc.sync.dma_start(out=bq_sb, in_=bq_d[:])
        nc.sync.dma_start(out=bk_sb, in_=bk_d[:])
        nc.sync.dma_start(out=tri, in_=tri_d[:])
        for dt_ in range(NDT):
            load_xt(dt_, 0)
        nc.sync.dma_start(out=wv_sb[:].bitcast(F32R), in_=wv_r)
        load_w_cols(wq_sb, wq_r, 1)
        load_w_cols(wk_sb, wk_r, 1)
        load_w_cols(wq_sb, wq_r, 2)
        load_w_cols(wk_sb, wk_r, 2)
        nc.sync.dma_start(
            out=xT[:, :, QB:S].bitcast(F32R), in_=xt_r[:, :, QB:S]
        )
        nc.sync.dma_start(out=wo_sb[:].bitcast(F32R), in_=wo_r)

        # ---- persistent activations ----
        qT = persist.tile([P, NPAIR, S], F32)     # Q^T, head pairs stacked
        kT = persist.tile([P, NPAIR, S], F32)
        vA = persist.tile([P, NST, NHC, H + 1], F32)  # V + ones col, per k-tile
        zT = persist.tile([P, NPAIR, S], F32)     # z^T (normalized), pairs stacked

        nc.gpsimd.memset(vA[:, :, :, H : H + 1], 1.0)

        ps_proj = ctx.enter_context(tc.tile_pool(name="ps_proj", bufs=2, space="PSUM"))
        ps_s = ctx.enter_context(tc.tile_pool(name="ps_s", bufs=1, space="PSUM"))
        ps_z = ctx.enter_context(tc.tile_pool(name="ps_z", bufs=1, space="PSUM"))
        ps_o = ctx.enter_context(tc.tile_pool(name="ps_o", bufs=2, space="PSUM"))

        def proj_qk(g, s2):
            qps = ps_proj.tile([P, QB], F32, tag="qk")
            for dt_ in range(NDT):
                nc.tensor.matmul(
                    qps,
                    _r(wq_sb[:, dt_, g * P : (g + 1) * P]),
                    _r(xT[:, dt_, s2 * QB : (s2 + 1) * QB]),
                    start=(dt_ == 0),
                    stop=(dt_ == NDT - 1),
                )
            nc.scalar.add(
                qT[:, g, s2 * QB : (s2 + 1) * QB].bitcast(F32R),
                qps,
                bq_sb[:, g : g + 1],
            )
            kps = ps_proj.tile([P, QB], F32, tag="qk")
            for dt_ in range(NDT):
                nc.tensor.matmul(
                    kps,
                    _r(wk_sb[:, dt_, g * P : (g + 1) * P]),
                    _r(xT[:, dt_, s2 * QB : (s2 + 1) * QB]),
                    start=(dt_ == 0),
                    stop=(dt_ == NDT - 1),
                )
            nc.scalar.add(
                kT[:, g, s2 * QB : (s2 + 1) * QB].bitcast(F32R),
                kps,
                bk_sb[:, g : g + 1],
            )

        def proj_v(st):
            vps = ps_proj.tile([P, HD], F32, tag="qk")
            for dt_ in range(NDT):
                nc.tensor.matmul(
                    vps,
                    _r(xT[:, dt_, st * P : (st + 1) * P]),
                    _r(wv_sb[:, dt_, :]),
                    start=(dt_ == 0),
                    stop=(dt_ == NDT - 1),
                )
            nc.vector.tensor_copy(
                out=vA[:, st, :, 0:H].bitcast(F32R),
                in_=vps.rearrange("p (n h) -> p n h", n=NHC),
            )

        def attend_pair(g, qb):
            """Both heads of pair g: the two K=64 score matmuls are packed
            into disjoint row-strips of the PE array via tile_position, so
            they run concurrently on the 32x32 sub-arrays."""
            q0 = qb * QB
            nkt = (qb + 1) * QB // P  # causal: k-tiles 0..nkt-1
            zzps = [
                ps_z.tile([H + 1, QB], F32, name=f"zps{hh}", tag=f"z{hh}")
                for hh in range(2)
            ]
            for kt in range(nkt):
                o = max(kt * P - q0, 0)  # first live column
                ets = []
                for hh in range(2):
                    hp = hh * H
                    sps = ps_s.tile([P, QB], F32, name=f"sps{hh}", tag=f"s{hh}")
                    nc.tensor.matmul(
                        sps[:, o:QB],
                        _r(kT[hp : hp + H, g, kt * P : (kt + 1) * P]),
                        _r(qT[hp : hp + H, g, q0 + o : q0 + QB]),
                        start=True,
                        stop=True,
                        tile_position=(hp, 0),
                    )
                    et = etp.tile([P, QB], F32)
                    nc.scalar.activation(
                        et[:, o:QB].bitcast(F32R),
                        sps[:, o:QB],
                        mybir.ActivationFunctionType.Exp,
                        scale=EXP_SCALE,
                    )
                    if kt * P - q0 >= -(P - 1):  # diagonal tile: partial block
                        eng = nc.vector if hh == 0 else nc.gpsimd
                        eng.tensor_mul(
                            et[:, o : o + P].bitcast(F32R), et[:, o : o + P], tri
                        )
                    ets.append(et)
                for hh in range(2):
                    nc.tensor.matmul(
                        zzps[hh][:, o:QB],
                        _r(vA[:, kt, 2 * g + hh, :]),
                        _r(ets[hh][:, o:QB]),
                        start=(kt == 0),
                        stop=(kt == nkt - 1),
                    )
            for hh in range(2):
                hp = hh * H
                zps = zzps[hh]
                # normalize: r = 1/l, broadcast over 64 partitions (gpsimd)
                r = smalls.tile([1, QB], F32)
                nc.vector.reciprocal(r, zps[H : H + 1, :])
                rb = smalls.tile([H, QB], F32, tag="rb")
                nc.gpsimd.partition_broadcast(rb, r)
                nc.vector.tensor_mul(
                    zT[hp : hp + H, g, q0 : q0 + QB].bitcast(F32R),
                    zps[0:H, :],
                    rb,
                )

        def out_proj(qb):
            q0 = qb * QB
            for qt in range(QB // P):
                row0 = q0 + qt * P
                out_t = outp.tile([P, D], F32)
                for dh in range(2):
                    ops = ps_o.tile([P, D // 2], F32)
                    for g in range(NPAIR):
                        nc.tensor.matmul(
                            ops,
                            _r(zT[:, g, row0 : row0 + P]),
                            _r(wo_sb[:, g, dh * (D // 2) : (dh + 1) * (D // 2)]),
                            start=(g == 0),
                            stop=(g == NPAIR - 1),
                        )
                    nc.vector.tensor_copy(
                        out=out_t[:, dh * (D // 2) : (dh + 1) * (D // 2)], in_=ops
                    )
                nc.sync.dma_start(out=out_d[row0 : row0 + P, :], in_=out_t)

        # phase 1+2: s2=0 projections pair-interleaved with qb0 attention
        proj_qk(0, 0)
        for st in range(4):
            proj_v(st)
        attend_pair(0, 0)
        proj_qk(1, 0)
        attend_pair(1, 0)
        proj_qk(2, 0)
        attend_pair(2, 0)
        out_proj(0)
        # phase 3: s2=1 projections, then qb1 attention
        for g in range(NPAIR):
            proj_qk(g, 1)
        for st in range(4, NST):
            proj_v(st)
        for g in range(NPAIR):
            attend_pair(g, 1)
        out_proj(1)

    if not nc.is_finalized():
        nc.finalize()
    return nc


def _get_program():
    if "nc" not in _CACHE:
        _CACHE["nc"] = _build()
    return _CACHE["nc"]


def make_in_maps(
    normalized_resid_pre, W_Q, W_K, W_V, W_O, b_Q, b_K, b_V=None, b_O=None, **_unused
):
    x = np.asarray(normalized_resid_pre, np.float32)
    W_Q, W_K, W_V = (np.asarray(a, np.float32) for a in (W_Q, W_K, W_V))
    W_O = np.asarray(W_O, np.float32)
    b_Q, b_K = np.asarray(b_Q, np.float32), np.asarray(b_K, np.float32)

    tri = np.triu(np.ones((P, P), np.float32))
    in_maps = []
    for c in range(8):
        b, hg = divmod(c, 2)
        hs = slice(hg * NHC, (hg + 1) * NHC)
        in_maps.append(
            {
                "xt": np.ascontiguousarray(x[b].T),
                "wq": np.ascontiguousarray(
                    W_Q[hs].transpose(1, 0, 2).reshape(D, HD)
                ),
                "wk": np.ascontiguousarray(
                    W_K[hs].transpose(1, 0, 2).reshape(D, HD)
                ),
                "wv": np.ascontiguousarray(
                    W_V[hs].transpose(1, 0, 2).reshape(D, HD)
                ),
                "wo": np.ascontiguousarray(W_O[hs].reshape(HD, D)),
                "bq": np.ascontiguousarray(b_Q[hs].reshape(NPAIR, P).T),
                "bk": np.ascontiguousarray(b_K[hs].reshape(NPAIR, P).T),
                "trimask": tri,
            }
        )
    return in_maps


def kernel(
    normalized_resid_pre, W_Q, W_K, W_V, W_O, b_Q, b_K, b_V, b_O, **_unused
):
    W_O = np.asarray(W_O, np.float32)
    b_V, b_O = np.asarray(b_V, np.float32), np.asarray(b_O, np.float32)
    in_maps = make_in_maps(
        normalized_resid_pre, W_Q, W_K, W_V, W_O, b_Q, b_K
    )

    nc = _get_program()
    res = run_bass_kernel_spmd(nc, in_maps, list(range(8))).results

    out = np.zeros((B, S, D), np.float32)
    for c in range(8):
        out[c // 2] += res[c]["out"]
    out += b_O + np.einsum("nh,nhd->d", b_V, W_O)
    return out
